# revision 17
# baseline (speedup 1.0000x reference)
"""AUAttnProcessor Trainium2 kernel.

Data-parallel over batch: 8 batch elements -> 8 NeuronCores, one full
attention-processor per core. Host does sharding + layout prep (transpose /
bf16 cast) only; all arithmetic runs on device.

Math (per batch element, weights stored [in, out]):
  q = hs @ Wq;  k/v = ehs @ Wk/Wv;  auk/auv = au @ Wau_k/Wau_v   (10 heads, dh=64)
  out  = softmax(q k^T * sc) v
  mask = sigmoid(q auk^T * sc / (|T|+eps)) * prior[t] * 0.9 + 0.1
  auo  = mask @ auv
  y    = (out + g * auo) @ Wout + bout + hs

Device formulation:
  - feature-major q^T [640, 4096]; scores^T [77, t] per head
  - PV matmul uses lhsT = [v_h | ones64] so rows 64:128 of the PSUM tile hold
    sum(exp) replicated; normalization is one reciprocal + one multiply on the
    way out of PSUM (linearity: (E @ v) / s == (E / s) @ v columnwise).
  - AU branch: auo @ Wout == msig @ W_hat + 0.1-rank-1 term, with
    W_hat[16h:16h+16] = auv_h @ Wout[64h:64h+64]; msig = sigmoid * (g*0.9*prior[t])
    applied as a free-dim broadcast multiply; the 0.1 term and bout enter via a
    K=1 ones matmul into the output accumulator.
"""

import os
import sys

sys.path.insert(0, "/opt/trn_rl_repo")

import numpy as np
import ml_dtypes

import concourse.bass as bass
import concourse.mybir as mybir
import concourse.tile as tile
from concourse import bacc
from concourse.bass_utils import run_bass_kernel_spmd

BF16 = mybir.dt.bfloat16
F32 = mybir.dt.float32
AF = mybir.ActivationFunctionType

B, S, HID = 8, 4096, 640
KV, AU, CROSS = 77, 16, 768
HEADS, DH = 10, 64
SCALE = DH**-0.5
NCORES = 8

CHUNK = 1024                       # token chunk for the attention phase
NCHUNK = S // CHUNK
TT = 128                           # token tile for the output projection
SUB = 512                          # matmul free-dim (one PSUM bank of fp32)

LAST_EXEC_NS = None


def _nsegs(n):
    """Split [0, n) into <=SUB segments."""
    return [(o, min(SUB, n - o)) for o in range(0, n, SUB)]


def _build(sig_scale: float, ag01: float):
    nc = bacc.Bacc("TRN2", target_bir_lowering=False, debug=False)

    din = {}
    def dt_in(name, shape, dtype):
        din[name] = nc.dram_tensor(name, shape, dtype, kind="ExternalInput")
        return din[name]

    hsT_d = dt_in("hsT", [HID, S], BF16)
    hsr_d = dt_in("hsr", [S, HID], F32)
    ehsT_d = dt_in("ehsT", [CROSS, KV], BF16)
    auT_d = dt_in("auT", [CROSS, AU], BF16)
    wq_d = dt_in("wq", [HID, HID], BF16)
    wk_d = dt_in("wk", [CROSS, HID], BF16)
    wv_d = dt_in("wv", [CROSS, HID], BF16)
    wauk_d = dt_in("wauk", [CROSS, HID], BF16)
    wauv_d = dt_in("wauv", [CROSS, HID], BF16)
    wout_d = dt_in("wout", [HID, HID], BF16)
    pv_d = dt_in("pv", [1, S], BF16)          # au_gate * 0.9 * prior
    bvec_d = dt_in("bvec", [1, HID], F32)     # bout
    y_d = nc.dram_tensor("y", [S, HID], F32, kind="ExternalOutput")

    KC5, KC6 = HID // 128, CROSS // 128       # contraction chunks

    from contextlib import ExitStack
    with tile.TileContext(nc) as tc, ExitStack() as stk:
        consts = stk.enter_context(tc.tile_pool(name="consts", bufs=1))
        ps_main = stk.enter_context(tc.tile_pool(name="ps_main", bufs=2, space="PSUM"))
        ps_work = stk.enter_context(tc.tile_pool(name="ps_work", bufs=4, space="PSUM"))


        dma = nc.sync.dma_start

        # ---- constant loads -------------------------------------------------
        ehsT = consts.tile([128, KC6, KV], BF16, tag="ehsT")
        dma(ehsT[:], ehsT_d.ap().rearrange("(c p) k -> p c k", p=128))
        auT = consts.tile([128, KC6, AU], BF16, tag="auT")
        dma(auT[:], auT_d.ap().rearrange("(c p) k -> p c k", p=128))
        wq = consts.tile([128, KC5, HID], BF16, tag="wq")
        dma(wq[:], wq_d.ap().rearrange("(c p) n -> p c n", p=128))
        wout = consts.tile([128, KC5, HID], BF16, tag="wout")
        dma(wout[:], wout_d.ap().rearrange("(c p) n -> p c n", p=128))

        pvbc = consts.tile([128, S], BF16, tag="pvbc")
        dma(pvbc[:], bass.AP(pv_d, 0, [[0, 128], [1, S]]))
        bvec = consts.tile([1, HID], F32, tag="bvec")
        dma(bvec[:], bvec_d.ap())

        ones1 = consts.tile([1, 128], BF16, tag="ones1")
        nc.vector.memset(ones1[:], 1.0)

        # ---- small projections ---------------------------------------------
        kT = consts.tile([128, KC5, KV], BF16, tag="kT")       # k^T [640, 77]
        aukT = consts.tile([128, KC5, AU], BF16, tag="aukT")   # auk^T [640, 16]
        auvT = consts.tile([128, KC5, AU], BF16, tag="auvT")   # auv^T [640, 16]
        vhat = consts.tile([KV, HEADS, DH], BF16, tag="vhat")  # v per head
        whatA = consts.tile([128, HID], BF16, tag="whatA")     # heads 0..7
        whatB = consts.tile([32, HID], BF16, tag="whatB")      # heads 8..9
        bias_bf = consts.tile([1, HID], BF16, tag="bias_bf")

        ones77 = consts.tile([KV, 1], BF16, tag="ones77")
        nc.vector.memset(ones77[:], 1.0)
        ident = consts.tile([128, 128], F32, tag="ident")
        from concourse.masks import make_identity
        make_identity(nc, ident[:])

        with tc.tile_pool(name="wsmall", bufs=1) as wsmall:
            wk = wsmall.tile([128, KC6, HID], BF16, tag="wk")
            dma(wk[:], wk_d.ap().rearrange("(c p) n -> p c n", p=128))
            wv = wsmall.tile([128, KC6, HID], BF16, tag="wv")
            dma(wv[:], wv_d.ap().rearrange("(c p) n -> p c n", p=128))
            wauk = wsmall.tile([128, KC6, HID], BF16, tag="wauk")
            dma(wauk[:], wauk_d.ap().rearrange("(c p) n -> p c n", p=128))
            wauv = wsmall.tile([128, KC6, HID], BF16, tag="wauv")
            dma(wauv[:], wauv_d.ap().rearrange("(c p) n -> p c n", p=128))

            # k^T / auk^T / auv^T: weights stationary, fout tiles of 128
            for c in range(KC5):
                for (w_sb, rhs_sb, dst, n) in (
                    (wk, ehsT, kT, KV),
                    (wauk, auT, aukT, AU),
                    (wauv, auT, auvT, AU),
                ):
                    ps = ps_work.tile([128, SUB], F32, tag="ps_work")
                    for kc in range(KC6):
                        nc.tensor.matmul(
                            ps[:, :n],
                            w_sb[:, kc, c * 128:(c + 1) * 128],
                            rhs_sb[:, kc, :],
                            start=(kc == 0), stop=(kc == KC6 - 1),
                        )
                    nc.vector.tensor_copy(dst[:, c, :], ps[:, :n])

            # v natural [77, 640] -> packed into vhat[:, h, 0:64]
            for off, n in _nsegs(HID):
                ps = ps_work.tile([128, SUB], F32, tag="ps_work")
                for kc in range(KC6):
                    nc.tensor.matmul(
                        ps[:KV, :n],
                        ehsT[:, kc, :],
                        wv[:, kc, off:off + n],
                        start=(kc == 0), stop=(kc == KC6 - 1),
                    )
                for h in range(off // DH, (off + n) // DH):
                    nc.vector.tensor_copy(
                        vhat[:, h, 0:DH], ps[:KV, h * DH - off:(h + 1) * DH - off]
                    )

        # W_hat[16h:16h+16, :] = auv_h @ Wout[64h:64h+64, :]
        # (engine partition access must be 32-aligned; route the 16-row
        # placement through a base-0 temp + SBUF->SBUF DMA)
        for h in range(HEADS):
            ps = ps_main.tile([128, HID], F32, tag="ps_main")
            r0 = (h % 2) * 64
            c = h // 2
            for off, n in _nsegs(HID):
                nc.tensor.matmul(
                    ps[:AU, off:off + n],
                    auvT[r0:r0 + 64, c, :],
                    wout[r0:r0 + 64, c, off:off + n],
                    start=True, stop=True,
                )
            wtmp = consts.tile([AU, HID], BF16, tag="wtmp")
            nc.vector.tensor_copy(wtmp[:], ps[:AU, :])
            dst = whatA[16 * h:16 * h + 16, :] if h < 8 else whatB[16 * (h - 8):16 * (h - 8) + 16, :]
            dma(dst, wtmp[:])

        # bias row: bout + ag01 * (sum_kv auv) @ Wout
        rsum = consts.tile([128, KC5], F32, tag="rsum")
        rsum_bf = consts.tile([128, KC5], BF16, tag="rsum_bf")
        for c in range(KC5):
            nc.vector.reduce_sum(rsum[:, c:c + 1], auvT[:, c, :], axis=mybir.AxisListType.X)
        nc.vector.tensor_copy(rsum_bf[:], rsum[:])
        ps_b = ps_main.tile([128, HID], F32, tag="ps_main")
        for off, n in _nsegs(HID):
            for c in range(KC5):
                nc.tensor.matmul(
                    ps_b[:1, off:off + n],
                    rsum_bf[:, c:c + 1],
                    wout[:, c, off:off + n],
                    start=(c == 0), stop=(c == KC5 - 1),
                )
        bias_f = consts.tile([1, HID], F32, tag="bias_f")
        nc.vector.tensor_scalar_mul(bias_f[:], ps_b[:1, :], ag01)
        nc.vector.tensor_add(bias_f[:], bias_f[:], bvec[:])
        nc.vector.tensor_copy(bias_bf[:], bias_f[:])

        # ---- q^T = Wq^T @ hs^T  (feature-major) ----------------------------
        qT = consts.tile([128, KC5, S], BF16, tag="qT")
        with tc.tile_pool(name="hsT", bufs=1) as hsT_pool:
            hsT = hsT_pool.tile([128, KC5, S], BF16, tag="hsT")
            dma(hsT[:], hsT_d.ap().rearrange("(c p) t -> p c t", p=128))
            for c in range(KC5):
                for t0 in range(0, S, SUB):
                    ps = ps_work.tile([128, SUB], F32, tag="ps_work")
                    for kc in range(KC5):
                        nc.tensor.matmul(
                            ps[:],
                            wq[:, kc, c * 128:(c + 1) * 128],
                            hsT[:, kc, t0:t0 + SUB],
                            start=(kc == 0), stop=(kc == KC5 - 1),
                        )
                    nc.scalar.copy(qT[:, c, t0:t0 + SUB], ps[:])

        # ---- attention + output projection, chunked over tokens ------------
        esc_pool = stk.enter_context(tc.tile_pool(name="esc", bufs=3))
        rec_pool = stk.enter_context(tc.tile_pool(name="rec", bufs=2))
        sig_pool = stk.enter_context(tc.tile_pool(name="sig", bufs=2))
        att_pool = stk.enter_context(tc.tile_pool(name="att", bufs=2))
        res_pool = stk.enter_context(tc.tile_pool(name="res", bufs=9))
        y_pool = stk.enter_context(tc.tile_pool(name="y", bufs=3))

        # ---- AU branch, hoisted: one Sigmoid table load total --------------
        sigA = consts.tile([128, S], BF16, tag="sigA")
        sigB = consts.tile([32, S], BF16, tag="sigB")
        for h in range(HEADS):
            r0 = (h % 2) * 64
            hc = h // 2
            sig_tmp = sig_pool.tile([AU, S], BF16, tag="sig_tmp")
            for s0 in range(0, S, SUB):
                ps_a = ps_work.tile([128, SUB], F32, tag="ps_work")
                nc.tensor.matmul(
                    ps_a[:AU, :],
                    aukT[r0:r0 + 64, hc, :],
                    qT[r0:r0 + 64, hc, s0:s0 + SUB],
                    start=True, stop=True,
                )
                nc.scalar.activation(
                    sig_tmp[:, s0:s0 + SUB], ps_a[:AU, :], AF.Sigmoid, scale=sig_scale
                )
            sg = sigA[16 * h:16 * h + 16, :] if h < 8 else \
                sigB[16 * (h - 8):16 * (h - 8) + 16, :]
            dma(sg, sig_tmp[:])
        # msig = sigmoid * (au_gate * 0.9 * prior[t]), in place
        nc.vector.tensor_mul(sigA[:], sigA[:], pvbc[:])
        nc.vector.tensor_mul(sigB[:], sigB[:], pvbc[:32, :])

        for ci in range(NCHUNK):
            c0 = ci * CHUNK
            attnT = att_pool.tile([128, KC5, CHUNK], BF16, tag="attnT")

            for j in range(HEADS // 2):
                hc = j
                escA = esc_pool.tile([KV, CHUNK], BF16, tag="escA")
                escB = esc_pool.tile([KV, CHUNK], BF16, tag="escB")

                # QK for the head pair: rows 0:64 and 64:128 pack on the array
                for s0 in range(0, CHUNK, SUB):
                    psA = ps_work.tile([128, SUB], F32, tag="ps_work")
                    psB = ps_work.tile([128, SUB], F32, tag="ps_work")
                    nc.tensor.matmul(
                        psA[:KV, :], kT[0:64, hc, :],
                        qT[0:64, hc, c0 + s0:c0 + s0 + SUB],
                        start=True, stop=True,
                    )
                    nc.tensor.matmul(
                        psB[:KV, :], kT[64:128, hc, :],
                        qT[64:128, hc, c0 + s0:c0 + s0 + SUB],
                        start=True, stop=True,
                    )
                    nc.scalar.activation(
                        escA[:, s0:s0 + SUB], psA[:KV, :], AF.Exp, scale=SCALE
                    )
                    nc.scalar.activation(
                        escB[:, s0:s0 + SUB], psB[:KV, :], AF.Exp, scale=SCALE
                    )

                for (h, esc, r0) in ((2 * j, escA, 0), (2 * j + 1, escB, 64)):
                    # dense softmax sums -> reciprocal -> transpose -> broadcast
                    ps_sums = ps_work.tile([128, SUB], F32, tag="ps_work")
                    for tt in range(CHUNK // TT):
                        nc.tensor.matmul(
                            ps_sums[:, tt:tt + 1],
                            esc[:, tt * TT:(tt + 1) * TT],
                            ones77[:],
                            start=True, stop=True,
                        )
                    rdense = rec_pool.tile([128, CHUNK // TT], F32, tag="rdense")
                    nc.vector.reciprocal(rdense[:], ps_sums[:, :CHUNK // TT])
                    ps_t = ps_work.tile([128, SUB], F32, tag="ps_work")
                    nc.tensor.transpose(ps_t[:CHUNK // TT, :128], rdense[:], ident[:])
                    recipT = rec_pool.tile([CHUNK // TT, 128], F32, tag="recipT")
                    nc.vector.tensor_copy(recipT[:], ps_t[:CHUNK // TT, :128])
                    recipbc = rec_pool.tile([64, CHUNK], F32, tag="recipbc")
                    for si in range(CHUNK // SUB):
                        rt4 = rec_pool.tile([1, SUB], F32, tag=f"rT4_{si}")
                        dma(
                            rt4[:].rearrange("p (k r) -> p k r", r=128),
                            recipT[4 * si:4 * si + 4, :],
                        )
                        nc.gpsimd.partition_broadcast(
                            recipbc[:, si * SUB:(si + 1) * SUB], rt4[:]
                        )

                    for s0 in range(0, CHUNK, SUB):
                        ps_pv = ps_work.tile([128, SUB], F32, tag="ps_work")
                        nc.tensor.matmul(
                            ps_pv[:DH, :],
                            vhat[:, h, :],
                            esc[:, s0:s0 + SUB],
                            start=True, stop=True,
                        )
                        nc.vector.tensor_mul(
                            attnT[r0:r0 + 64, hc, s0:s0 + SUB],
                            ps_pv[0:DH, :],
                            recipbc[:, s0:s0 + SUB],
                        )

            # output projection per 128-token tile (residual prefetched first)
            res_tiles = []
            for tt in range(CHUNK // TT):
                t0 = tt * TT
                res = res_pool.tile([TT, HID], F32, tag="res")
                dma(res[:], hsr_d.ap()[c0 + t0:c0 + t0 + TT, :])
                res_tiles.append(res)
            for tt in range(CHUNK // TT):
                t0 = tt * TT
                ps_y = ps_main.tile([128, HID], F32, tag="ps_main")
                for off, n in _nsegs(HID):
                    for kc in range(KC5):
                        nc.tensor.matmul(
                            ps_y[:, off:off + n],
                            attnT[:, kc, t0:t0 + TT],
                            wout[:, kc, off:off + n],
                            start=(kc == 0), stop=False,
                        )
                    nc.tensor.matmul(
                        ps_y[:, off:off + n],
                        sigA[:, c0 + t0:c0 + t0 + TT],
                        whatA[:, off:off + n],
                        start=False, stop=False,
                    )
                    nc.tensor.matmul(
                        ps_y[:, off:off + n],
                        sigB[:, c0 + t0:c0 + t0 + TT],
                        whatB[:, off:off + n],
                        start=False, stop=False,
                    )
                    nc.tensor.matmul(
                        ps_y[:, off:off + n],
                        ones1[:],
                        bias_bf[:, off:off + n],
                        start=False, stop=True,
                    )
                y_sb = y_pool.tile([TT, HID], F32, tag="y_sb")
                nc.vector.tensor_add(y_sb[:], ps_y[:], res_tiles[tt][:])
                dma(y_d.ap()[c0 + t0:c0 + t0 + TT, :], y_sb[:])

    nc.compile()
    return nc


_CACHE = {}


def _get_nc(sig_scale, ag01):
    key = (round(float(sig_scale), 12), round(float(ag01), 12))
    if key not in _CACHE:
        _CACHE[key] = _build(float(sig_scale), float(ag01))
    return _CACHE[key]


def _prior():
    lin = np.linspace(-1.0, 1.0, 64)
    yy, xx = np.meshgrid(lin, lin, indexing="ij")
    g = np.exp(-(xx**2 + yy**2) / (2 * 0.55**2))
    return g.reshape(-1).astype(np.float32)


def kernel(hidden_states, encoder_hidden_states, au_embedding, Wq, Wk, Wv,
           Wau_k, Wau_v, Wout, bout, temperature, au_gate):
    global LAST_EXEC_NS
    bf = ml_dtypes.bfloat16

    hs = np.asarray(hidden_states, dtype=np.float32)
    ehs = np.asarray(encoder_hidden_states, dtype=np.float32)
    au = np.asarray(au_embedding, dtype=np.float32)
    temp = float(np.abs(np.asarray(temperature).reshape(-1)[0])) + 1e-6
    ag = float(np.asarray(au_gate).reshape(-1)[0])

    sig_scale = SCALE / temp
    ag01 = ag * 0.1
    nc = _get_nc(sig_scale, ag01)

    pvec = (ag * 0.9 * _prior()).reshape(1, S).astype(bf)
    shared = {
        "wq": np.asarray(Wq, np.float32).astype(bf),
        "wk": np.asarray(Wk, np.float32).astype(bf),
        "wv": np.asarray(Wv, np.float32).astype(bf),
        "wauk": np.asarray(Wau_k, np.float32).astype(bf),
        "wauv": np.asarray(Wau_v, np.float32).astype(bf),
        "wout": np.asarray(Wout, np.float32).astype(bf),
        "pv": pvec,
        "bvec": np.asarray(bout, np.float32).reshape(1, HID),
    }
    in_maps = []
    for b in range(B):
        m = dict(shared)
        m["hsT"] = np.ascontiguousarray(hs[b].T).astype(bf)
        m["hsr"] = np.ascontiguousarray(hs[b])
        m["ehsT"] = np.ascontiguousarray(ehs[b].T).astype(bf)
        m["auT"] = np.ascontiguousarray(au[b].T).astype(bf)
        in_maps.append(m)

    trace = bool(os.environ.get("KERNEL_TRACE"))
    if trace:
        try:
            import trace_shim
            trace_shim.install()
        except Exception:
            pass
    res = run_bass_kernel_spmd(nc, in_maps, core_ids=list(range(NCORES)), trace=trace)
    LAST_EXEC_NS = res.exec_time_ns

    out = np.stack([res.results[i]["y"] for i in range(B)]).astype(np.float32)
    return out


# revision 20
# speedup vs baseline: 1.0932x; 1.0932x over previous
"""AUAttnProcessor Trainium2 kernel.

Data-parallel over batch: 8 batch elements -> 8 NeuronCores, one full
attention-processor per core. Host does sharding + layout prep (transpose /
bf16 cast) only; all arithmetic runs on device.

Math (per batch element, weights stored [in, out]):
  q = hs @ Wq;  k/v = ehs @ Wk/Wv;  auk/auv = au @ Wau_k/Wau_v   (10 heads, dh=64)
  out  = softmax(q k^T * sc) v
  mask = sigmoid(q auk^T * sc / (|T|+eps)) * prior[t] * 0.9 + 0.1
  auo  = mask @ auv
  y    = (out + g * auo) @ Wout + bout + hs

Device formulation:
  - feature-major q^T [640, 4096]; scores^T [77, t] per head
  - PV matmul uses lhsT = [v_h | ones64] so rows 64:128 of the PSUM tile hold
    sum(exp) replicated; normalization is one reciprocal + one multiply on the
    way out of PSUM (linearity: (E @ v) / s == (E / s) @ v columnwise).
  - AU branch: auo @ Wout == msig @ W_hat + 0.1-rank-1 term, with
    W_hat[16h:16h+16] = auv_h @ Wout[64h:64h+64]; msig = sigmoid * (g*0.9*prior[t])
    applied as a free-dim broadcast multiply; the 0.1 term and bout enter via a
    K=1 ones matmul into the output accumulator.
"""

import os
import sys

sys.path.insert(0, "/opt/trn_rl_repo")

import numpy as np
import ml_dtypes

import concourse.bass as bass
import concourse.mybir as mybir
import concourse.tile as tile
from concourse import bacc
from concourse.bass_utils import run_bass_kernel_spmd

BF16 = mybir.dt.bfloat16
F32 = mybir.dt.float32
AF = mybir.ActivationFunctionType

B, S, HID = 8, 4096, 640
KV, AU, CROSS = 77, 16, 768
HEADS, DH = 10, 64
SCALE = DH**-0.5
NCORES = 8

CHUNK = 2048                       # token chunk for the attention phase
NCHUNK = S // CHUNK
TT = 128                           # token tile for the output projection
SUB = 512                          # matmul free-dim (one PSUM bank of fp32)

LAST_EXEC_NS = None


def _nsegs(n):
    """Split [0, n) into <=SUB segments."""
    return [(o, min(SUB, n - o)) for o in range(0, n, SUB)]


def _build(sig_scale: float, ag01: float):
    nc = bacc.Bacc("TRN2", target_bir_lowering=False, debug=False)

    din = {}
    def dt_in(name, shape, dtype):
        din[name] = nc.dram_tensor(name, shape, dtype, kind="ExternalInput")
        return din[name]

    hsT_d = dt_in("hsT", [HID, S], BF16)
    hsr_d = dt_in("hsr", [S, HID], F32)
    ehsT_d = dt_in("ehsT", [CROSS, KV], BF16)
    auT_d = dt_in("auT", [CROSS, AU], BF16)
    wq_d = dt_in("wq", [HID, HID], BF16)
    wk_d = dt_in("wk", [CROSS, HID], BF16)
    wv_d = dt_in("wv", [CROSS, HID], BF16)
    wauk_d = dt_in("wauk", [CROSS, HID], BF16)
    wauv_d = dt_in("wauv", [CROSS, HID], BF16)
    wout_d = dt_in("wout", [HID, HID], BF16)
    pv_d = dt_in("pv", [1, S], BF16)          # au_gate * 0.9 * prior
    bvec_d = dt_in("bvec", [1, HID], F32)     # bout
    y_d = nc.dram_tensor("y", [S, HID], F32, kind="ExternalOutput")

    KC5, KC6 = HID // 128, CROSS // 128       # contraction chunks

    from contextlib import ExitStack
    with tile.TileContext(nc) as tc, ExitStack() as stk:
        consts = stk.enter_context(tc.tile_pool(name="consts", bufs=1))
        ps_main = stk.enter_context(tc.tile_pool(name="ps_main", bufs=1, space="PSUM"))
        ps_work = stk.enter_context(tc.tile_pool(name="ps_work", bufs=5, space="PSUM"))


        dma = nc.sync.dma_start

        # ---- constant loads -------------------------------------------------
        ehsT = consts.tile([128, KC6, KV], BF16, tag="ehsT")
        dma(ehsT[:], ehsT_d.ap().rearrange("(c p) k -> p c k", p=128))
        auT = consts.tile([128, KC6, AU], BF16, tag="auT")
        dma(auT[:], auT_d.ap().rearrange("(c p) k -> p c k", p=128))
        wq = consts.tile([128, KC5, HID], BF16, tag="wq")
        dma(wq[:], wq_d.ap().rearrange("(c p) n -> p c n", p=128))
        wout = consts.tile([128, KC5, HID], BF16, tag="wout")
        dma(wout[:], wout_d.ap().rearrange("(c p) n -> p c n", p=128))

        pvbc = consts.tile([128, S], BF16, tag="pvbc")
        dma(pvbc[:], bass.AP(pv_d, 0, [[0, 128], [1, S]]))
        bvec = consts.tile([1, HID], F32, tag="bvec")
        dma(bvec[:], bvec_d.ap())

        ones1 = consts.tile([1, 128], BF16, tag="ones1")
        nc.vector.memset(ones1[:], 1.0)

        # ---- small projections ---------------------------------------------
        kT = consts.tile([128, KC5, KV], BF16, tag="kT")       # k^T [640, 77]
        aukT = consts.tile([128, KC5, AU], BF16, tag="aukT")   # auk^T [640, 16]
        auvT = consts.tile([128, KC5, AU], BF16, tag="auvT")   # auv^T [640, 16]
        vhat = consts.tile([KV, HEADS, DH], BF16, tag="vhat")  # v per head
        whatA = consts.tile([128, HID], BF16, tag="whatA")     # heads 0..7
        whatB = consts.tile([32, HID], BF16, tag="whatB")      # heads 8..9
        bias_bf = consts.tile([1, HID], BF16, tag="bias_bf")

        ones77 = consts.tile([KV, 1], BF16, tag="ones77")
        nc.vector.memset(ones77[:], 1.0)
        ident = consts.tile([128, 128], F32, tag="ident")
        from concourse.masks import make_identity
        make_identity(nc, ident[:])

        with tc.tile_pool(name="wsmall", bufs=1) as wsmall:
            wk = wsmall.tile([128, KC6, HID], BF16, tag="wk")
            dma(wk[:], wk_d.ap().rearrange("(c p) n -> p c n", p=128))
            wv = wsmall.tile([128, KC6, HID], BF16, tag="wv")
            dma(wv[:], wv_d.ap().rearrange("(c p) n -> p c n", p=128))
            wauk = wsmall.tile([128, KC6, HID], BF16, tag="wauk")
            dma(wauk[:], wauk_d.ap().rearrange("(c p) n -> p c n", p=128))
            wauv = wsmall.tile([128, KC6, HID], BF16, tag="wauv")
            dma(wauv[:], wauv_d.ap().rearrange("(c p) n -> p c n", p=128))

            # k^T / auk^T / auv^T: weights stationary, fout tiles of 128
            for c in range(KC5):
                for (w_sb, rhs_sb, dst, n) in (
                    (wk, ehsT, kT, KV),
                    (wauk, auT, aukT, AU),
                    (wauv, auT, auvT, AU),
                ):
                    ps = ps_work.tile([128, SUB], F32, tag="ps_work")
                    for kc in range(KC6):
                        nc.tensor.matmul(
                            ps[:, :n],
                            w_sb[:, kc, c * 128:(c + 1) * 128],
                            rhs_sb[:, kc, :],
                            start=(kc == 0), stop=(kc == KC6 - 1),
                        )
                    nc.vector.tensor_copy(dst[:, c, :], ps[:, :n])

            # v natural [77, 640] -> packed into vhat[:, h, 0:64]
            for off, n in _nsegs(HID):
                ps = ps_work.tile([128, SUB], F32, tag="ps_work")
                for kc in range(KC6):
                    nc.tensor.matmul(
                        ps[:KV, :n],
                        ehsT[:, kc, :],
                        wv[:, kc, off:off + n],
                        start=(kc == 0), stop=(kc == KC6 - 1),
                    )
                for h in range(off // DH, (off + n) // DH):
                    nc.vector.tensor_copy(
                        vhat[:, h, 0:DH], ps[:KV, h * DH - off:(h + 1) * DH - off]
                    )

        # W_hat[16h:16h+16, :] = auv_h @ Wout[64h:64h+64, :]
        # (engine partition access must be 32-aligned; route the 16-row
        # placement through a base-0 temp + SBUF->SBUF DMA)
        for h in range(HEADS):
            ps = ps_main.tile([128, HID], F32, tag="ps_main")
            r0 = (h % 2) * 64
            c = h // 2
            for off, n in _nsegs(HID):
                nc.tensor.matmul(
                    ps[:AU, off:off + n],
                    auvT[r0:r0 + 64, c, :],
                    wout[r0:r0 + 64, c, off:off + n],
                    start=True, stop=True,
                )
            wtmp = consts.tile([AU, HID], BF16, tag="wtmp")
            nc.vector.tensor_copy(wtmp[:], ps[:AU, :])
            dst = whatA[16 * h:16 * h + 16, :] if h < 8 else whatB[16 * (h - 8):16 * (h - 8) + 16, :]
            dma(dst, wtmp[:])

        # bias row: bout + ag01 * (sum_kv auv) @ Wout
        rsum = consts.tile([128, KC5], F32, tag="rsum")
        rsum_bf = consts.tile([128, KC5], BF16, tag="rsum_bf")
        for c in range(KC5):
            nc.vector.reduce_sum(rsum[:, c:c + 1], auvT[:, c, :], axis=mybir.AxisListType.X)
        nc.vector.tensor_copy(rsum_bf[:], rsum[:])
        ps_b = ps_main.tile([128, HID], F32, tag="ps_main")
        for off, n in _nsegs(HID):
            for c in range(KC5):
                nc.tensor.matmul(
                    ps_b[:1, off:off + n],
                    rsum_bf[:, c:c + 1],
                    wout[:, c, off:off + n],
                    start=(c == 0), stop=(c == KC5 - 1),
                )
        bias_f = consts.tile([1, HID], F32, tag="bias_f")
        nc.vector.tensor_scalar_mul(bias_f[:], ps_b[:1, :], ag01)
        nc.vector.tensor_add(bias_f[:], bias_f[:], bvec[:])
        nc.vector.tensor_copy(bias_bf[:], bias_f[:])

        # ---- q^T = Wq^T @ hs^T  (feature-major) ----------------------------
        qT = consts.tile([128, KC5, S], BF16, tag="qT")
        with tc.tile_pool(name="hsT", bufs=1) as hsT_pool:
            hsT = hsT_pool.tile([128, KC5, S], BF16, tag="hsT")
            dma(hsT[:], hsT_d.ap().rearrange("(c p) t -> p c t", p=128))
            for c in range(KC5):
                for t0 in range(0, S, SUB):
                    ps = ps_work.tile([128, SUB], F32, tag="ps_work")
                    for kc in range(KC5):
                        nc.tensor.matmul(
                            ps[:],
                            wq[:, kc, c * 128:(c + 1) * 128],
                            hsT[:, kc, t0:t0 + SUB],
                            start=(kc == 0), stop=(kc == KC5 - 1),
                        )
                    nc.scalar.copy(qT[:, c, t0:t0 + SUB], ps[:])

        # ---- attention + output projection, chunked over tokens ------------
        esc_pool = stk.enter_context(tc.tile_pool(name="esc", bufs=2))
        rec_pool = stk.enter_context(tc.tile_pool(name="rec", bufs=2))
        sig_pool = stk.enter_context(tc.tile_pool(name="sig", bufs=2))
        att_pool = stk.enter_context(tc.tile_pool(name="att", bufs=2))
        res_pool = stk.enter_context(tc.tile_pool(name="res", bufs=6))
        y_pool = stk.enter_context(tc.tile_pool(name="y", bufs=3))

        # ---- AU branch, hoisted: one Sigmoid table load total --------------
        sigA = consts.tile([128, S], BF16, tag="sigA")
        sigB = consts.tile([32, S], BF16, tag="sigB")
        for h in range(HEADS):
            r0 = (h % 2) * 64
            hc = h // 2
            sig_tmp = sig_pool.tile([AU, S], BF16, tag="sig_tmp")
            for s0 in range(0, S, SUB):
                ps_a = ps_work.tile([128, SUB], F32, tag="ps_work")
                nc.tensor.matmul(
                    ps_a[:AU, :],
                    aukT[r0:r0 + 64, hc, :],
                    qT[r0:r0 + 64, hc, s0:s0 + SUB],
                    start=True, stop=True,
                )
                nc.scalar.activation(
                    sig_tmp[:, s0:s0 + SUB], ps_a[:AU, :], AF.Sigmoid, scale=sig_scale
                )
            sg = sigA[16 * h:16 * h + 16, :] if h < 8 else \
                sigB[16 * (h - 8):16 * (h - 8) + 16, :]
            dma(sg, sig_tmp[:])
        # msig = sigmoid * (au_gate * 0.9 * prior[t]), in place
        nc.vector.tensor_mul(sigA[:], sigA[:], pvbc[:])
        nc.vector.tensor_mul(sigB[:], sigB[:], pvbc[:32, :])

        for ci in range(NCHUNK):
            c0 = ci * CHUNK
            attnT = att_pool.tile([128, KC5, CHUNK], BF16, tag="attnT")

            for j in range(HEADS // 2):
                hc = j
                escA = esc_pool.tile([KV, CHUNK], BF16, tag="escA")
                escB = esc_pool.tile([KV, CHUNK], BF16, tag="escB")

                # QK for the head pair: rows 0:64 and 64:128 pack on the array
                for s0 in range(0, CHUNK, SUB):
                    psA = ps_work.tile([128, SUB], F32, tag="ps_work")
                    psB = ps_work.tile([128, SUB], F32, tag="ps_work")
                    nc.tensor.matmul(
                        psA[:KV, :], kT[0:64, hc, :],
                        qT[0:64, hc, c0 + s0:c0 + s0 + SUB],
                        start=True, stop=True,
                    )
                    nc.tensor.matmul(
                        psB[:KV, :], kT[64:128, hc, :],
                        qT[64:128, hc, c0 + s0:c0 + s0 + SUB],
                        start=True, stop=True,
                    )
                    nc.scalar.activation(
                        escA[:, s0:s0 + SUB], psA[:KV, :], AF.Exp, scale=SCALE
                    )
                    nc.scalar.activation(
                        escB[:, s0:s0 + SUB], psB[:KV, :], AF.Exp, scale=SCALE
                    )

                for (h, esc, r0) in ((2 * j, escA, 0), (2 * j + 1, escB, 64)):
                    # dense softmax sums -> reciprocal -> transpose -> broadcast
                    ps_sums = ps_work.tile([128, SUB], F32, tag="ps_work")
                    for tt in range(CHUNK // TT):
                        nc.tensor.matmul(
                            ps_sums[:, tt:tt + 1],
                            esc[:, tt * TT:(tt + 1) * TT],
                            ones77[:],
                            start=True, stop=True,
                        )
                    rdense = rec_pool.tile([128, CHUNK // TT], F32, tag="rdense")
                    nc.vector.reciprocal(rdense[:], ps_sums[:, :CHUNK // TT])
                    ps_t = ps_work.tile([128, SUB], F32, tag="ps_work")
                    nc.tensor.transpose(ps_t[:CHUNK // TT, :128], rdense[:], ident[:])
                    recipT = rec_pool.tile([CHUNK // TT, 128], BF16, tag="recipT")
                    nc.vector.tensor_copy(recipT[:], ps_t[:CHUNK // TT, :128])
                    recipbc = rec_pool.tile([64, CHUNK], BF16, tag="recipbc")
                    rt4 = rec_pool.tile([1, CHUNK], BF16, tag="rT4")
                    dma(
                        rt4[:].rearrange("p (k r) -> p k r", r=128),
                        recipT[:],
                    )
                    nc.gpsimd.partition_broadcast(recipbc[:], rt4[:])

                    for s0 in range(0, CHUNK, SUB):
                        ps_pv = ps_work.tile([128, SUB], F32, tag="ps_work")
                        nc.tensor.matmul(
                            ps_pv[:DH, :],
                            vhat[:, h, :],
                            esc[:, s0:s0 + SUB],
                            start=True, stop=True,
                        )
                        nc.vector.tensor_mul(
                            attnT[r0:r0 + 64, hc, s0:s0 + SUB],
                            ps_pv[0:DH, :],
                            recipbc[:, s0:s0 + SUB],
                        )

            # output projection per 128-token tile (residual prefetched first)
            res_tiles = {}
            def _res_prefetch(tt):
                if tt < CHUNK // TT and tt not in res_tiles:
                    res = res_pool.tile([TT, HID], F32, tag="res")
                    dma(res[:], hsr_d.ap()[c0 + tt * TT:c0 + (tt + 1) * TT, :])
                    res_tiles[tt] = res
            for tt in range(4):
                _res_prefetch(tt)
            for tt in range(CHUNK // TT):
                t0 = tt * TT
                _res_prefetch(tt + 4)
                y_sb = y_pool.tile([TT, HID], F32, tag="y_sb")
                for off, n in _nsegs(HID):
                    ps_y = ps_work.tile([128, SUB], F32, tag="ps_work")
                    for kc in range(KC5):
                        nc.tensor.matmul(
                            ps_y[:, :n],
                            attnT[:, kc, t0:t0 + TT],
                            wout[:, kc, off:off + n],
                            start=(kc == 0), stop=False,
                        )
                    nc.tensor.matmul(
                        ps_y[:, :n],
                        sigA[:, c0 + t0:c0 + t0 + TT],
                        whatA[:, off:off + n],
                        start=False, stop=False,
                    )
                    nc.tensor.matmul(
                        ps_y[:, :n],
                        sigB[:, c0 + t0:c0 + t0 + TT],
                        whatB[:, off:off + n],
                        start=False, stop=False,
                    )
                    nc.tensor.matmul(
                        ps_y[:, :n],
                        ones1[:],
                        bias_bf[:, off:off + n],
                        start=False, stop=True,
                    )
                    nc.vector.tensor_add(
                        y_sb[:, off:off + n], ps_y[:, :n], res_tiles[tt][:, off:off + n]
                    )
                dma(y_d.ap()[c0 + t0:c0 + t0 + TT, :], y_sb[:])

    nc.compile()
    return nc


_CACHE = {}


def _get_nc(sig_scale, ag01):
    key = (round(float(sig_scale), 12), round(float(ag01), 12))
    if key not in _CACHE:
        _CACHE[key] = _build(float(sig_scale), float(ag01))
    return _CACHE[key]


def _prior():
    lin = np.linspace(-1.0, 1.0, 64)
    yy, xx = np.meshgrid(lin, lin, indexing="ij")
    g = np.exp(-(xx**2 + yy**2) / (2 * 0.55**2))
    return g.reshape(-1).astype(np.float32)


def kernel(hidden_states, encoder_hidden_states, au_embedding, Wq, Wk, Wv,
           Wau_k, Wau_v, Wout, bout, temperature, au_gate):
    global LAST_EXEC_NS
    bf = ml_dtypes.bfloat16

    hs = np.asarray(hidden_states, dtype=np.float32)
    ehs = np.asarray(encoder_hidden_states, dtype=np.float32)
    au = np.asarray(au_embedding, dtype=np.float32)
    temp = float(np.abs(np.asarray(temperature).reshape(-1)[0])) + 1e-6
    ag = float(np.asarray(au_gate).reshape(-1)[0])

    sig_scale = SCALE / temp
    ag01 = ag * 0.1
    nc = _get_nc(sig_scale, ag01)

    pvec = (ag * 0.9 * _prior()).reshape(1, S).astype(bf)
    shared = {
        "wq": np.asarray(Wq, np.float32).astype(bf),
        "wk": np.asarray(Wk, np.float32).astype(bf),
        "wv": np.asarray(Wv, np.float32).astype(bf),
        "wauk": np.asarray(Wau_k, np.float32).astype(bf),
        "wauv": np.asarray(Wau_v, np.float32).astype(bf),
        "wout": np.asarray(Wout, np.float32).astype(bf),
        "pv": pvec,
        "bvec": np.asarray(bout, np.float32).reshape(1, HID),
    }
    in_maps = []
    for b in range(B):
        m = dict(shared)
        m["hsT"] = np.ascontiguousarray(hs[b].T).astype(bf)
        m["hsr"] = np.ascontiguousarray(hs[b])
        m["ehsT"] = np.ascontiguousarray(ehs[b].T).astype(bf)
        m["auT"] = np.ascontiguousarray(au[b].T).astype(bf)
        in_maps.append(m)

    trace = bool(os.environ.get("KERNEL_TRACE"))
    if trace:
        try:
            import trace_shim
            trace_shim.install()
        except Exception:
            pass
    res = run_bass_kernel_spmd(nc, in_maps, core_ids=list(range(NCORES)), trace=trace)
    LAST_EXEC_NS = res.exec_time_ns

    out = np.stack([res.results[i]["y"] for i in range(B)]).astype(np.float32)
    return out


# revision 21
# speedup vs baseline: 1.2556x; 1.1485x over previous
"""AUAttnProcessor Trainium2 kernel.

Data-parallel over batch: 8 batch elements -> 8 NeuronCores, one full
attention-processor per core. Host does sharding + layout prep (transpose /
bf16 cast) only; all arithmetic runs on device.

Math (per batch element, weights stored [in, out]):
  q = hs @ Wq;  k/v = ehs @ Wk/Wv;  auk/auv = au @ Wau_k/Wau_v   (10 heads, dh=64)
  out  = softmax(q k^T * sc) v
  mask = sigmoid(q auk^T * sc / (|T|+eps)) * prior[t] * 0.9 + 0.1
  auo  = mask @ auv
  y    = (out + g * auo) @ Wout + bout + hs

Device formulation:
  - feature-major q^T [640, 4096]; scores^T [77, t] per head
  - PV matmul uses lhsT = [v_h | ones64] so rows 64:128 of the PSUM tile hold
    sum(exp) replicated; normalization is one reciprocal + one multiply on the
    way out of PSUM (linearity: (E @ v) / s == (E / s) @ v columnwise).
  - AU branch: auo @ Wout == msig @ W_hat + 0.1-rank-1 term, with
    W_hat[16h:16h+16] = auv_h @ Wout[64h:64h+64]; msig = sigmoid * (g*0.9*prior[t])
    applied as a free-dim broadcast multiply; the 0.1 term and bout enter via a
    K=1 ones matmul into the output accumulator.
"""

import os
import sys

sys.path.insert(0, "/opt/trn_rl_repo")

import numpy as np
import ml_dtypes

import concourse.bass as bass
import concourse.mybir as mybir
import concourse.tile as tile
from concourse import bacc
from concourse.bass_utils import run_bass_kernel_spmd

BF16 = mybir.dt.bfloat16
F32 = mybir.dt.float32
AF = mybir.ActivationFunctionType

B, S, HID = 8, 4096, 640
KV, AU, CROSS = 77, 16, 768
HEADS, DH = 10, 64
SCALE = DH**-0.5
NCORES = 8

CHUNK = 2048                       # token chunk for the attention phase
NCHUNK = S // CHUNK
TT = 128                           # token tile for the output projection
SUB = 512                          # matmul free-dim (one PSUM bank of fp32)

LAST_EXEC_NS = None


def _nsegs(n):
    """Split [0, n) into <=SUB segments."""
    return [(o, min(SUB, n - o)) for o in range(0, n, SUB)]


def _build(sig_scale: float, ag01: float):
    nc = bacc.Bacc("TRN2", target_bir_lowering=False, debug=False)

    din = {}
    def dt_in(name, shape, dtype):
        din[name] = nc.dram_tensor(name, shape, dtype, kind="ExternalInput")
        return din[name]

    hsT_d = dt_in("hsT", [HID, S], BF16)
    hsr_d = dt_in("hsr", [S, HID], F32)
    ehsT_d = dt_in("ehsT", [CROSS, KV], BF16)
    auT_d = dt_in("auT", [CROSS, AU], BF16)
    wq_d = dt_in("wq", [HID, HID], BF16)
    wk_d = dt_in("wk", [CROSS, HID], BF16)
    wv_d = dt_in("wv", [CROSS, HID], BF16)
    wauk_d = dt_in("wauk", [CROSS, HID], BF16)
    wauv_d = dt_in("wauv", [CROSS, HID], BF16)
    wout_d = dt_in("wout", [HID, HID], BF16)
    pv_d = dt_in("pv", [1, S], BF16)          # au_gate * 0.9 * prior
    bvec_d = dt_in("bvec", [1, HID], F32)     # bout
    y_d = nc.dram_tensor("y", [S, HID], F32, kind="ExternalOutput")

    KC5, KC6 = HID // 128, CROSS // 128       # contraction chunks

    from contextlib import ExitStack
    with tile.TileContext(nc) as tc, ExitStack() as stk:
        consts = stk.enter_context(tc.tile_pool(name="consts", bufs=1))
        ps_work = stk.enter_context(tc.tile_pool(name="ps_work", bufs=7, space="PSUM"))


        dma = nc.sync.dma_start

        # ---- constant loads -------------------------------------------------
        ehsT = consts.tile([128, KC6, KV], BF16, tag="ehsT")
        dma(ehsT[:], ehsT_d.ap().rearrange("(c p) k -> p c k", p=128))
        auT = consts.tile([128, KC6, AU], BF16, tag="auT")
        dma(auT[:], auT_d.ap().rearrange("(c p) k -> p c k", p=128))
        wq = consts.tile([128, KC5, HID], BF16, tag="wq")
        dma(wq[:], wq_d.ap().rearrange("(c p) n -> p c n", p=128))
        wout = consts.tile([128, KC5, HID], BF16, tag="wout")
        dma(wout[:], wout_d.ap().rearrange("(c p) n -> p c n", p=128))

        pvbc = consts.tile([128, S], BF16, tag="pvbc")
        dma(pvbc[:], bass.AP(pv_d, 0, [[0, 128], [1, S]]))
        bvec = consts.tile([1, HID], F32, tag="bvec")
        dma(bvec[:], bvec_d.ap())

        ones1 = consts.tile([1, 128], BF16, tag="ones1")
        nc.vector.memset(ones1[:], 1.0)

        # ---- small projections ---------------------------------------------
        kT = consts.tile([128, KC5, KV], BF16, tag="kT")       # k^T [640, 77]
        aukT = consts.tile([128, KC5, AU], BF16, tag="aukT")   # auk^T [640, 16]
        auvT = consts.tile([128, KC5, AU], BF16, tag="auvT")   # auv^T [640, 16]
        vhat = consts.tile([KV, HEADS, DH], BF16, tag="vhat")  # v per head
        whatA = consts.tile([128, HID], BF16, tag="whatA")     # heads 0..7
        whatB = consts.tile([32, HID], BF16, tag="whatB")      # heads 8..9
        bias_bf = consts.tile([1, HID], BF16, tag="bias_bf")

        ones77 = consts.tile([KV, 1], BF16, tag="ones77")
        nc.vector.memset(ones77[:], 1.0)
        ident = consts.tile([128, 128], F32, tag="ident")
        from concourse.masks import make_identity
        make_identity(nc, ident[:])

        with tc.tile_pool(name="wsmall", bufs=1) as wsmall:
            wk = wsmall.tile([128, KC6, HID], BF16, tag="wk")
            dma(wk[:], wk_d.ap().rearrange("(c p) n -> p c n", p=128))
            wv = wsmall.tile([128, KC6, HID], BF16, tag="wv")
            dma(wv[:], wv_d.ap().rearrange("(c p) n -> p c n", p=128))
            wauk = wsmall.tile([128, KC6, HID], BF16, tag="wauk")
            dma(wauk[:], wauk_d.ap().rearrange("(c p) n -> p c n", p=128))
            wauv = wsmall.tile([128, KC6, HID], BF16, tag="wauv")
            dma(wauv[:], wauv_d.ap().rearrange("(c p) n -> p c n", p=128))

            # k^T / auk^T / auv^T: weights stationary, fout tiles of 128
            for c in range(KC5):
                for (w_sb, rhs_sb, dst, n) in (
                    (wk, ehsT, kT, KV),
                    (wauk, auT, aukT, AU),
                    (wauv, auT, auvT, AU),
                ):
                    ps = ps_work.tile([128, SUB], F32, tag="ps_work")
                    for kc in range(KC6):
                        nc.tensor.matmul(
                            ps[:, :n],
                            w_sb[:, kc, c * 128:(c + 1) * 128],
                            rhs_sb[:, kc, :],
                            start=(kc == 0), stop=(kc == KC6 - 1),
                        )
                    nc.vector.tensor_copy(dst[:, c, :], ps[:, :n])

            # v natural [77, 640] -> packed into vhat[:, h, 0:64]
            for off, n in _nsegs(HID):
                ps = ps_work.tile([128, SUB], F32, tag="ps_work")
                for kc in range(KC6):
                    nc.tensor.matmul(
                        ps[:KV, :n],
                        ehsT[:, kc, :],
                        wv[:, kc, off:off + n],
                        start=(kc == 0), stop=(kc == KC6 - 1),
                    )
                for h in range(off // DH, (off + n) // DH):
                    nc.vector.tensor_copy(
                        vhat[:, h, 0:DH], ps[:KV, h * DH - off:(h + 1) * DH - off]
                    )

        # W_hat[16h:16h+16, :] = auv_h @ Wout[64h:64h+64, :]
        # (engine partition access must be 32-aligned; route the 16-row
        # placement through a base-0 temp + SBUF->SBUF DMA)
        for h in range(HEADS):
            r0 = (h % 2) * 64
            c = h // 2
            wtmp = consts.tile([AU, HID], BF16, tag="wtmp")
            for off, n in _nsegs(HID):
                ps = ps_work.tile([128, SUB], F32, tag="ps_work")
                nc.tensor.matmul(
                    ps[:AU, :n],
                    auvT[r0:r0 + 64, c, :],
                    wout[r0:r0 + 64, c, off:off + n],
                    start=True, stop=True,
                )
                nc.vector.tensor_copy(wtmp[:, off:off + n], ps[:AU, :n])
            dst = whatA[16 * h:16 * h + 16, :] if h < 8 else whatB[16 * (h - 8):16 * (h - 8) + 16, :]
            dma(dst, wtmp[:])

        # bias row: bout + ag01 * (sum_kv auv) @ Wout
        rsum = consts.tile([128, KC5], F32, tag="rsum")
        rsum_bf = consts.tile([128, KC5], BF16, tag="rsum_bf")
        for c in range(KC5):
            nc.vector.reduce_sum(rsum[:, c:c + 1], auvT[:, c, :], axis=mybir.AxisListType.X)
        nc.vector.tensor_copy(rsum_bf[:], rsum[:])
        bias_f = consts.tile([1, HID], F32, tag="bias_f")
        for off, n in _nsegs(HID):
            ps_b = ps_work.tile([128, SUB], F32, tag="ps_work")
            for c in range(KC5):
                nc.tensor.matmul(
                    ps_b[:1, :n],
                    rsum_bf[:, c:c + 1],
                    wout[:, c, off:off + n],
                    start=(c == 0), stop=(c == KC5 - 1),
                )
            nc.vector.tensor_scalar_mul(bias_f[:, off:off + n], ps_b[:1, :n], ag01)
        nc.vector.tensor_add(bias_f[:], bias_f[:], bvec[:])
        nc.vector.tensor_copy(bias_bf[:], bias_f[:])

        # ---- q^T = Wq^T @ hs^T  (feature-major) ----------------------------
        qT = consts.tile([128, KC5, S], BF16, tag="qT")
        with tc.tile_pool(name="hsT", bufs=1) as hsT_pool:
            hsT = hsT_pool.tile([128, KC5, S], BF16, tag="hsT")
            dma(hsT[:], hsT_d.ap().rearrange("(c p) t -> p c t", p=128))
            for c in range(KC5):
                for t0 in range(0, S, SUB):
                    ps = ps_work.tile([128, SUB], F32, tag="ps_work")
                    for kc in range(KC5):
                        nc.tensor.matmul(
                            ps[:],
                            wq[:, kc, c * 128:(c + 1) * 128],
                            hsT[:, kc, t0:t0 + SUB],
                            start=(kc == 0), stop=(kc == KC5 - 1),
                        )
                    nc.scalar.copy(qT[:, c, t0:t0 + SUB], ps[:])

        # ---- attention + output projection, chunked over tokens ------------
        esc_pool = stk.enter_context(tc.tile_pool(name="esc", bufs=2))
        rec_pool = stk.enter_context(tc.tile_pool(name="rec", bufs=2))
        sig_pool = stk.enter_context(tc.tile_pool(name="sig", bufs=2))
        att_pool = stk.enter_context(tc.tile_pool(name="att", bufs=2))
        res_pool = stk.enter_context(tc.tile_pool(name="res", bufs=6))
        y_pool = stk.enter_context(tc.tile_pool(name="y", bufs=3))

        # ---- AU branch, hoisted: one Sigmoid table load total --------------
        sigA = consts.tile([128, S], BF16, tag="sigA")
        sigB = consts.tile([32, S], BF16, tag="sigB")
        for h in range(HEADS):
            r0 = (h % 2) * 64
            hc = h // 2
            sig_tmp = sig_pool.tile([AU, S], BF16, tag="sig_tmp")
            for s0 in range(0, S, SUB):
                ps_a = ps_work.tile([128, SUB], F32, tag="ps_work")
                nc.tensor.matmul(
                    ps_a[:AU, :],
                    aukT[r0:r0 + 64, hc, :],
                    qT[r0:r0 + 64, hc, s0:s0 + SUB],
                    start=True, stop=True,
                )
                nc.scalar.activation(
                    sig_tmp[:, s0:s0 + SUB], ps_a[:AU, :], AF.Sigmoid, scale=sig_scale
                )
            sg = sigA[16 * h:16 * h + 16, :] if h < 8 else \
                sigB[16 * (h - 8):16 * (h - 8) + 16, :]
            dma(sg, sig_tmp[:])
        # msig = sigmoid * (au_gate * 0.9 * prior[t]), in place
        nc.vector.tensor_mul(sigA[:], sigA[:], pvbc[:])
        nc.vector.tensor_mul(sigB[:], sigB[:], pvbc[:32, :])

        for ci in range(NCHUNK):
            c0 = ci * CHUNK
            attnT = att_pool.tile([128, KC5, CHUNK], BF16, tag="attnT")

            for j in range(HEADS // 2):
                hc = j
                escA = esc_pool.tile([KV, CHUNK], BF16, tag="escA")
                escB = esc_pool.tile([KV, CHUNK], BF16, tag="escB")

                # QK for the head pair: rows 0:64 and 64:128 pack on the array
                for s0 in range(0, CHUNK, SUB):
                    psA = ps_work.tile([128, SUB], F32, tag="ps_work")
                    psB = ps_work.tile([128, SUB], F32, tag="ps_work")
                    nc.tensor.matmul(
                        psA[:KV, :], kT[0:64, hc, :],
                        qT[0:64, hc, c0 + s0:c0 + s0 + SUB],
                        start=True, stop=True,
                    )
                    nc.tensor.matmul(
                        psB[:KV, :], kT[64:128, hc, :],
                        qT[64:128, hc, c0 + s0:c0 + s0 + SUB],
                        start=True, stop=True,
                    )
                    nc.scalar.activation(
                        escA[:, s0:s0 + SUB], psA[:KV, :], AF.Exp, scale=SCALE
                    )
                    nc.scalar.activation(
                        escB[:, s0:s0 + SUB], psB[:KV, :], AF.Exp, scale=SCALE
                    )

                for (h, esc, r0) in ((2 * j, escA, 0), (2 * j + 1, escB, 64)):
                    # dense softmax sums -> reciprocal -> transpose -> broadcast
                    ps_sums = ps_work.tile([128, SUB], F32, tag="ps_work")
                    for tt in range(CHUNK // TT):
                        nc.tensor.matmul(
                            ps_sums[:, tt:tt + 1],
                            esc[:, tt * TT:(tt + 1) * TT],
                            ones77[:],
                            start=True, stop=True,
                        )
                    rdense = rec_pool.tile([128, CHUNK // TT], F32, tag="rdense")
                    nc.vector.reciprocal(rdense[:], ps_sums[:, :CHUNK // TT])
                    ps_t = ps_work.tile([128, SUB], F32, tag="ps_work")
                    nc.tensor.transpose(ps_t[:CHUNK // TT, :128], rdense[:], ident[:])
                    recipT = rec_pool.tile([CHUNK // TT, 128], BF16, tag="recipT")
                    nc.vector.tensor_copy(recipT[:], ps_t[:CHUNK // TT, :128])
                    recipbc = rec_pool.tile([64, CHUNK], BF16, tag="recipbc")
                    rt4 = rec_pool.tile([1, CHUNK], BF16, tag="rT4")
                    dma(
                        rt4[:].rearrange("p (k r) -> p k r", r=128),
                        recipT[:],
                    )
                    nc.gpsimd.partition_broadcast(recipbc[:], rt4[:])

                    for s0 in range(0, CHUNK, SUB):
                        ps_pv = ps_work.tile([128, SUB], F32, tag="ps_work")
                        nc.tensor.matmul(
                            ps_pv[:DH, :],
                            vhat[:, h, :],
                            esc[:, s0:s0 + SUB],
                            start=True, stop=True,
                        )
                        nc.vector.tensor_mul(
                            attnT[r0:r0 + 64, hc, s0:s0 + SUB],
                            ps_pv[0:DH, :],
                            recipbc[:, s0:s0 + SUB],
                        )

            # output projection per 128-token tile (residual prefetched first)
            res_tiles = {}
            def _res_prefetch(tt):
                if tt < CHUNK // TT and tt not in res_tiles:
                    res = res_pool.tile([TT, HID], F32, tag="res")
                    dma(res[:], hsr_d.ap()[c0 + tt * TT:c0 + (tt + 1) * TT, :])
                    res_tiles[tt] = res
            for tt in range(4):
                _res_prefetch(tt)
            for tt in range(CHUNK // TT):
                t0 = tt * TT
                _res_prefetch(tt + 4)
                y_sb = y_pool.tile([TT, HID], F32, tag="y_sb")
                for off, n in _nsegs(HID):
                    ps_y = ps_work.tile([128, SUB], F32, tag="ps_work")
                    for kc in range(KC5):
                        nc.tensor.matmul(
                            ps_y[:, :n],
                            attnT[:, kc, t0:t0 + TT],
                            wout[:, kc, off:off + n],
                            start=(kc == 0), stop=False,
                        )
                    nc.tensor.matmul(
                        ps_y[:, :n],
                        sigA[:, c0 + t0:c0 + t0 + TT],
                        whatA[:, off:off + n],
                        start=False, stop=False,
                    )
                    nc.tensor.matmul(
                        ps_y[:, :n],
                        sigB[:, c0 + t0:c0 + t0 + TT],
                        whatB[:, off:off + n],
                        start=False, stop=False,
                    )
                    nc.tensor.matmul(
                        ps_y[:, :n],
                        ones1[:],
                        bias_bf[:, off:off + n],
                        start=False, stop=True,
                    )
                    nc.vector.tensor_add(
                        y_sb[:, off:off + n], ps_y[:, :n], res_tiles[tt][:, off:off + n]
                    )
                dma(y_d.ap()[c0 + t0:c0 + t0 + TT, :], y_sb[:])

    nc.compile()
    return nc


_CACHE = {}


def _get_nc(sig_scale, ag01):
    key = (round(float(sig_scale), 12), round(float(ag01), 12))
    if key not in _CACHE:
        _CACHE[key] = _build(float(sig_scale), float(ag01))
    return _CACHE[key]


def _prior():
    lin = np.linspace(-1.0, 1.0, 64)
    yy, xx = np.meshgrid(lin, lin, indexing="ij")
    g = np.exp(-(xx**2 + yy**2) / (2 * 0.55**2))
    return g.reshape(-1).astype(np.float32)


def kernel(hidden_states, encoder_hidden_states, au_embedding, Wq, Wk, Wv,
           Wau_k, Wau_v, Wout, bout, temperature, au_gate):
    global LAST_EXEC_NS
    bf = ml_dtypes.bfloat16

    hs = np.asarray(hidden_states, dtype=np.float32)
    ehs = np.asarray(encoder_hidden_states, dtype=np.float32)
    au = np.asarray(au_embedding, dtype=np.float32)
    temp = float(np.abs(np.asarray(temperature).reshape(-1)[0])) + 1e-6
    ag = float(np.asarray(au_gate).reshape(-1)[0])

    sig_scale = SCALE / temp
    ag01 = ag * 0.1
    nc = _get_nc(sig_scale, ag01)

    pvec = (ag * 0.9 * _prior()).reshape(1, S).astype(bf)
    shared = {
        "wq": np.asarray(Wq, np.float32).astype(bf),
        "wk": np.asarray(Wk, np.float32).astype(bf),
        "wv": np.asarray(Wv, np.float32).astype(bf),
        "wauk": np.asarray(Wau_k, np.float32).astype(bf),
        "wauv": np.asarray(Wau_v, np.float32).astype(bf),
        "wout": np.asarray(Wout, np.float32).astype(bf),
        "pv": pvec,
        "bvec": np.asarray(bout, np.float32).reshape(1, HID),
    }
    in_maps = []
    for b in range(B):
        m = dict(shared)
        m["hsT"] = np.ascontiguousarray(hs[b].T).astype(bf)
        m["hsr"] = np.ascontiguousarray(hs[b])
        m["ehsT"] = np.ascontiguousarray(ehs[b].T).astype(bf)
        m["auT"] = np.ascontiguousarray(au[b].T).astype(bf)
        in_maps.append(m)

    trace = bool(os.environ.get("KERNEL_TRACE"))
    if trace:
        try:
            import trace_shim
            trace_shim.install()
        except Exception:
            pass
    res = run_bass_kernel_spmd(nc, in_maps, core_ids=list(range(NCORES)), trace=trace)
    LAST_EXEC_NS = res.exec_time_ns

    out = np.stack([res.results[i]["y"] for i in range(B)]).astype(np.float32)
    return out


# revision 28
# speedup vs baseline: 1.3932x; 1.1096x over previous
"""AUAttnProcessor Trainium2 kernel.

Data-parallel over batch: 8 batch elements -> 8 NeuronCores, one full
attention-processor per core. Host does sharding + layout prep (transpose /
bf16 cast) only; all arithmetic runs on device.

Math (per batch element, weights stored [in, out]):
  q = hs @ Wq;  k/v = ehs @ Wk/Wv;  auk/auv = au @ Wau_k/Wau_v   (10 heads, dh=64)
  out  = softmax(q k^T * sc) v
  mask = sigmoid(q auk^T * sc / (|T|+eps)) * prior[t] * 0.9 + 0.1
  auo  = mask @ auv
  y    = (out + g * auo) @ Wout + bout + hs

Device formulation (all matmuls bf16 with fp32 PSUM accumulation):
  - feature-major q^T [640, 4096]; per head-pair QK packs array row groups
    0:64 / 64:128 concurrently.
  - softmax denominators land TOKEN-major via tiny N=1 matmuls with the
    escore tile as the stationary operand; one cheap reciprocal per
    (head, chunk); a PE transpose + SBUF repack + gpsimd partition_broadcast
    turns them into a feature-major [64, chunk] operand.  Normalization rides
    the PV-evacuation multiply (linearity of the PV matmul).
  - AU branch: auo @ Wout == msig @ W_hat + rank-1 term, with
    W_hat[16h:16h+16] = auv_h @ Wout[64h:64h+64]; msig = sigmoid * (g*0.9*prior)
    via a free-dim broadcast multiply; the 0.1 term and bout fold into a
    per-partition bias column applied at evacuation.
  - output projection is feature-major (Wout stationary), y^T goes to DRAM
    and the host transposes on gather; bias + residual are fused into one
    DVE scalar_tensor_tensor per tile.
  - phase schedule keeps the PE dense (HAM-warm): q-quarters stream in as the
    DMA lands; W_hat/bias/AU-sigmoid builds fill chunk 0's softmax-chain
    stalls; chunk ci-1's output projection fills chunk ci's.
"""

import os
import sys

sys.path.insert(0, "/opt/trn_rl_repo")

import numpy as np
import ml_dtypes

import concourse.bass as bass
import concourse.mybir as mybir
import concourse.tile as tile
from concourse import bacc
from concourse.bass_utils import run_bass_kernel_spmd
from concourse.masks import make_identity

BF16 = mybir.dt.bfloat16
F32 = mybir.dt.float32
AF = mybir.ActivationFunctionType

B, S, HID = 8, 4096, 640
KV, AU, CROSS = 77, 16, 768
HEADS, DH = 10, 64
SCALE = DH**-0.5
NCORES = 8

CHUNK = 2048                # token chunk for the attention/out-proj pipeline
NCHUNK = S // CHUNK
TT = 128                    # token tile (dense softmax sums, matmul M limit)
SUB = 512                   # matmul free-dim (one PSUM bank of fp32)
KC5, KC6 = HID // 128, CROSS // 128

LAST_EXEC_NS = None


def _nsegs(n):
    return [(o, min(SUB, n - o)) for o in range(0, n, SUB)]


def _build(sig_scale: float, ag01: float):
    nc = bacc.Bacc("TRN2", target_bir_lowering=False, debug=False)

    def dt_in(name, shape, dtype):
        return nc.dram_tensor(name, shape, dtype, kind="ExternalInput")

    hsT_d = dt_in("hsT", [HID, S], BF16)
    hsrT_d = dt_in("hsrT", [HID, S], F32)
    ehsT_d = dt_in("ehsT", [CROSS, KV], BF16)
    auT_d = dt_in("auT", [CROSS, AU], BF16)
    wq_d = dt_in("wq", [HID, HID], BF16)
    wk_d = dt_in("wk", [CROSS, HID], BF16)
    wv_d = dt_in("wv", [CROSS, HID], BF16)
    wauk_d = dt_in("wauk", [CROSS, HID], BF16)
    wauv_d = dt_in("wauv", [CROSS, HID], BF16)
    wout_d = dt_in("wout", [HID, HID], BF16)
    pv_d = dt_in("pv", [1, S], BF16)                 # au_gate * 0.9 * prior
    bvecT_d = dt_in("bvecT", [128, KC5], F32)        # bout column-major
    y_d = nc.dram_tensor("y", [HID, S], F32, kind="ExternalOutput")  # y^T

    from contextlib import ExitStack
    with tile.TileContext(nc) as tc, ExitStack() as stk:
        consts = stk.enter_context(tc.tile_pool(name="consts", bufs=1))
        ps_work = stk.enter_context(tc.tile_pool(name="ps_work", bufs=7, space="PSUM"))

        dma = nc.sync.dma_start

        # ---- critical-path DMAs first: wq then hsT (token quarters) --------
        wq = consts.tile([128, KC5, HID], BF16, tag="wq")
        dma(wq[:], wq_d.ap().rearrange("(c p) n -> p c n", p=128))
        hsT_pool_cm = tc.tile_pool(name="hsT", bufs=1)
        hsT_pool = hsT_pool_cm.__enter__()
        hsT = hsT_pool.tile([128, KC5, S], BF16, tag="hsT")
        for qq in range(4):
            sl = slice(qq * (S // 4), (qq + 1) * (S // 4))
            dma(hsT[:, :, sl], hsT_d.ap().rearrange("(c p) t -> p c t", p=128)[:, :, sl])

        # ---- remaining input DMAs ------------------------------------------
        ehsT = consts.tile([128, KC6, KV], BF16, tag="ehsT")
        dma(ehsT[:], ehsT_d.ap().rearrange("(c p) k -> p c k", p=128))
        auT = consts.tile([128, KC6, AU], BF16, tag="auT")
        dma(auT[:], auT_d.ap().rearrange("(c p) k -> p c k", p=128))
        wout = consts.tile([128, KC5, HID], BF16, tag="wout")
        dma(wout[:], wout_d.ap().rearrange("(c p) n -> p c n", p=128))
        pvbc = consts.tile([128, S], BF16, tag="pvbc")
        dma(pvbc[:], bass.AP(pv_d, 0, [[0, 128], [1, S]]))
        bvecT = consts.tile([128, KC5], F32, tag="bvecT")
        dma(bvecT[:], bvecT_d.ap())

        # ---- small constants ------------------------------------------------
        kT = consts.tile([128, KC5, KV], BF16, tag="kT")
        aukT = consts.tile([128, KC5, AU], BF16, tag="aukT")
        auvT = consts.tile([128, KC5, AU], BF16, tag="auvT")
        vhat = consts.tile([KV, HEADS, DH], BF16, tag="vhat")
        whatA = consts.tile([128, HID], BF16, tag="whatA")
        whatB = consts.tile([32, HID], BF16, tag="whatB")
        bias_colT = consts.tile([128, KC5], F32, tag="bias_colT")
        sigA = consts.tile([128, S], BF16, tag="sigA")
        sigB = consts.tile([32, S], BF16, tag="sigB")
        ones77 = consts.tile([KV, 1], BF16, tag="ones77")
        nc.vector.memset(ones77[:], 1.0)
        ident = consts.tile([128, 128], F32, tag="ident")
        make_identity(nc, ident[:])

        # ---- q^T = Wq^T @ hs^T, streamed per DMA quarter --------------------
        qTs = [consts.tile([128, S], BF16, name=f"qT{c}", tag=f"qT{c}") for c in range(KC5)]
        for qq in range(4):
            for c in range(KC5):
                for t0 in range(qq * (S // 4), (qq + 1) * (S // 4), SUB):
                    ps = ps_work.tile([128, SUB], F32, name="ps_q", tag="ps_work")
                    for kc in range(KC5):
                        nc.tensor.matmul(
                            ps[:],
                            wq[:, kc, c * 128:(c + 1) * 128],
                            hsT[:, kc, t0:t0 + SUB],
                            start=(kc == 0), stop=(kc == KC5 - 1),
                        )
                    nc.scalar.copy(qTs[c][:, t0:t0 + SUB], ps[:])

        # ---- small projections (weights reuse the hsT slot, WAR-ordered) ---
        w4 = hsT_pool.tile([128, 4, KC6, HID], BF16, tag="hsT")
        wk, wv, wauk, wauv = (w4[:, i] for i in range(4))
        dma(wk[:], wk_d.ap().rearrange("(c p) n -> p c n", p=128))
        dma(wv[:], wv_d.ap().rearrange("(c p) n -> p c n", p=128))
        dma(wauk[:], wauk_d.ap().rearrange("(c p) n -> p c n", p=128))
        dma(wauv[:], wauv_d.ap().rearrange("(c p) n -> p c n", p=128))

        for c in range(KC5):
            for (w_sb, rhs_sb, dst, n) in (
                (wk, ehsT, kT, KV),
                (wauk, auT, aukT, AU),
                (wauv, auT, auvT, AU),
            ):
                ps = ps_work.tile([128, SUB], F32, name="ps_s", tag="ps_work")
                for kc in range(KC6):
                    nc.tensor.matmul(
                        ps[:, :n],
                        w_sb[:, kc, c * 128:(c + 1) * 128],
                        rhs_sb[:, kc, :],
                        start=(kc == 0), stop=(kc == KC6 - 1),
                    )
                nc.vector.tensor_copy(dst[:, c, :], ps[:, :n])

        for off, n in _nsegs(HID):
            ps = ps_work.tile([128, SUB], F32, name="ps_v", tag="ps_work")
            for kc in range(KC6):
                nc.tensor.matmul(
                    ps[:KV, :n],
                    ehsT[:, kc, :],
                    wv[:, kc, off:off + n],
                    start=(kc == 0), stop=(kc == KC6 - 1),
                )
            for h in range(off // DH, (off + n) // DH):
                nc.vector.tensor_copy(
                    vhat[:, h, 0:DH], ps[:KV, h * DH - off:(h + 1) * DH - off]
                )

        hsT_pool_cm.__exit__(None, None, None)

        # ---- pools for the attention pipeline (after the hsT slot is freed) -
        esc_pool = stk.enter_context(tc.tile_pool(name="esc", bufs=3))
        rec_pool = stk.enter_context(tc.tile_pool(name="rec", bufs=3))
        sig_pool = stk.enter_context(tc.tile_pool(name="sig", bufs=2))
        att_pool = stk.enter_context(tc.tile_pool(name="att", bufs=1))
        res_pool = stk.enter_context(tc.tile_pool(name="res", bufs=6))
        y_pool = stk.enter_context(tc.tile_pool(name="y", bufs=3))

        attnTs = [att_pool.tile([128, KC5, CHUNK], BF16, name=f"attnT{i}", tag=f"attnT{i}")
                  for i in range(2)]

        # ---- deferred builds: emitted as dense PE filler inside chunk 0 -----
        def build_what(h):
            r0 = (h % 2) * 64
            c = h // 2
            wtmp = consts.tile([AU, HID], BF16, name="wtmp", tag="wtmp")
            for off, n in _nsegs(HID):
                ps = ps_work.tile([128, SUB], F32, name="ps_w", tag="ps_work")
                nc.tensor.matmul(
                    ps[:AU, :n],
                    auvT[r0:r0 + 64, c, :],
                    wout[r0:r0 + 64, c, off:off + n],
                    start=True, stop=True,
                )
                nc.vector.tensor_copy(wtmp[:, off:off + n], ps[:AU, :n])
            dst = whatA[16 * h:16 * h + 16, :] if h < 8 else \
                whatB[16 * (h - 8):16 * (h - 8) + 16, :]
            dma(dst, wtmp[:])

        def build_bias():
            rsum = consts.tile([128, KC5], F32, tag="rsum")
            rsum_bf = consts.tile([128, KC5], BF16, tag="rsum_bf")
            for c in range(KC5):
                nc.vector.reduce_sum(rsum[:, c:c + 1], auvT[:, c, :], axis=mybir.AxisListType.X)
            nc.vector.tensor_copy(rsum_bf[:], rsum[:])
            for c in range(KC5):
                ps_b = ps_work.tile([128, SUB], F32, name="ps_b", tag="ps_work")
                for kc in range(KC5):
                    nc.tensor.matmul(
                        ps_b[:, 0:1],
                        wout[:, kc, c * 128:(c + 1) * 128],
                        rsum_bf[:, kc:kc + 1],
                        start=(kc == 0), stop=(kc == KC5 - 1),
                    )
                nc.vector.tensor_scalar_mul(bias_colT[:, c:c + 1], ps_b[:, 0:1], ag01)
            nc.vector.tensor_add(bias_colT[:], bias_colT[:], bvecT[:])

        def emit_au_group(g):
            # 4 heads per PSUM tile at 32-aligned column groups; one sigmoid
            # per tile (junk rows between heads are computed, never read)
            heads = list(range(4 * g, min(4 * g + 4, HEADS)))
            sig_tmp = sig_pool.tile([112, S], BF16, name="sig_tmp", tag="sig_tmp")
            for s0 in range(0, S, SUB):
                ps_a = ps_work.tile([128, SUB], F32, name="ps_a", tag="ps_work")
                for k, h in enumerate(heads):
                    r0 = (h % 2) * 64
                    nc.tensor.matmul(
                        ps_a[32 * k:32 * k + AU, :],
                        aukT[r0:r0 + 64, h // 2, :],
                        qTs[h // 2][r0:r0 + 64, s0:s0 + SUB],
                        start=True, stop=True,
                        tile_position=(r0, 32 * k),
                    )
                nc.scalar.activation(
                    sig_tmp[:32 * len(heads) - 16, s0:s0 + SUB],
                    ps_a[:32 * len(heads) - 16, :],
                    AF.Sigmoid, scale=sig_scale,
                )
            for k, h in enumerate(heads):
                sg = sigA[16 * h:16 * h + 16, :] if h < 8 else \
                    sigB[16 * (h - 8):16 * (h - 8) + 16, :]
                dma(sg, sig_tmp[32 * k:32 * k + 16, :])

        def emit_msig():
            nc.vector.tensor_mul(sigA[:], sigA[:], pvbc[:])
            nc.vector.tensor_mul(sigB[:], sigB[:], pvbc[:32, :])

        # ---- one output-projection c-sweep of chunk ci ----------------------
        def outproj_sweep(ci, c):
            c0 = ci * CHUNK
            attnT = attnTs[ci % 2]
            for si in range(CHUNK // SUB):
                t0 = si * SUB
                rt = res_pool.tile([128, SUB], F32, name="res", tag="res")
                dma(rt[:], hsrT_d.ap().rearrange("(c p) t -> c p t", p=128)
                    [c, :, c0 + t0:c0 + t0 + SUB])
                ps_y = ps_work.tile([128, SUB], F32, name="ps_y", tag="ps_work")
                for kc in range(KC5):
                    nc.tensor.matmul(
                        ps_y[:],
                        wout[:, kc, c * 128:(c + 1) * 128],
                        attnT[:, kc, t0:t0 + SUB],
                        start=(kc == 0), stop=False,
                    )
                nc.tensor.matmul(
                    ps_y[:], whatA[:, c * 128:(c + 1) * 128],
                    sigA[:, c0 + t0:c0 + t0 + SUB], start=False, stop=False,
                )
                nc.tensor.matmul(
                    ps_y[:], whatB[:, c * 128:(c + 1) * 128],
                    sigB[:, c0 + t0:c0 + t0 + SUB], start=False, stop=True,
                )
                y_sb = y_pool.tile([128, SUB], F32, name="y_sb", tag="y_sb")
                nc.vector.scalar_tensor_tensor(
                    out=y_sb[:], in0=ps_y[:], scalar=bias_colT[:, c:c + 1],
                    in1=rt[:], op0=mybir.AluOpType.add, op1=mybir.AluOpType.add,
                )
                dma(
                    y_d.ap().rearrange("(c p) t -> c p t", p=128)
                    [c, :, c0 + t0:c0 + t0 + SUB],
                    y_sb[:],
                )

        # ---- attention pipeline --------------------------------------------
        fillers = [lambda: emit_au_group(0), lambda: emit_au_group(1),
                   lambda: (emit_au_group(2), emit_msig())] + \
            [(lambda h=h: (build_what(2 * h), build_what(2 * h + 1))) for h in range(5)] + \
            [build_bias]

        for ci in range(NCHUNK):
            c0 = ci * CHUNK
            attnT = attnTs[ci % 2]

            def stage1(j, c0=c0):
                hc = j
                escA = esc_pool.tile([KV, CHUNK], BF16, name="escA", tag="escA")
                escB = esc_pool.tile([KV, CHUNK], BF16, name="escB", tag="escB")
                for s0 in range(0, CHUNK, SUB):
                    psA = ps_work.tile([128, SUB], F32, name="psA", tag="ps_work")
                    psB = ps_work.tile([128, SUB], F32, name="psB", tag="ps_work")
                    nc.tensor.matmul(
                        psA[:KV, :], kT[0:64, hc, :],
                        qTs[hc][0:64, c0 + s0:c0 + s0 + SUB],
                        start=True, stop=True,
                    )
                    nc.tensor.matmul(
                        psB[:KV, :], kT[64:128, hc, :],
                        qTs[hc][64:128, c0 + s0:c0 + s0 + SUB],
                        start=True, stop=True,
                    )
                    nc.scalar.activation(
                        escA[:, s0:s0 + SUB], psA[:KV, :], AF.Exp, scale=SCALE)
                    nc.scalar.activation(
                        escB[:, s0:s0 + SUB], psB[:KV, :], AF.Exp, scale=SCALE)
                recips = []
                for (h, esc) in ((2 * j, escA), (2 * j + 1, escB)):
                    ps_sums = ps_work.tile([128, SUB], F32, name="ps_sums", tag="ps_work")
                    for tt in range(CHUNK // TT):
                        nc.tensor.matmul(
                            ps_sums[:, tt:tt + 1],
                            esc[:, tt * TT:(tt + 1) * TT],
                            ones77[:],
                            start=True, stop=True,
                        )
                    rdense = rec_pool.tile([128, CHUNK // TT], F32, name="rdense", tag="rdense")
                    nc.vector.reciprocal(rdense[:], ps_sums[:, :CHUNK // TT])
                    ps_t = ps_work.tile([128, SUB], F32, name="ps_t", tag="ps_work")
                    nc.tensor.transpose(ps_t[:CHUNK // TT, :128], rdense[:], ident[:])
                    recipT = rec_pool.tile([CHUNK // TT, 128], BF16, name="recipT", tag="recipT")
                    nc.vector.tensor_copy(recipT[:], ps_t[:CHUNK // TT, :128])
                    recipbc = rec_pool.tile([64, CHUNK], BF16, name="recipbc", tag="recipbc")
                    rt4 = rec_pool.tile([1, CHUNK], BF16, name="rt4", tag="rT4")
                    dma(rt4[:].rearrange("p (k r) -> p k r", r=128), recipT[:])
                    nc.gpsimd.partition_broadcast(recipbc[:], rt4[:])
                    recips.append(recipbc)
                return (escA, escB, recips)

            def stage2(j, st, attnT=attnT, c0=c0):
                hc = j
                escA, escB, recips = st
                for (h, esc, r0, recipbc) in (
                    (2 * j, escA, 0, recips[0]),
                    (2 * j + 1, escB, 64, recips[1]),
                ):
                    for s0 in range(0, CHUNK, SUB):
                        ps_pv = ps_work.tile([128, SUB], F32, name="ps_pv", tag="ps_work")
                        nc.tensor.matmul(
                            ps_pv[:DH, :],
                            vhat[:, h, :],
                            esc[:, s0:s0 + SUB],
                            start=True, stop=True,
                        )
                        nc.vector.tensor_mul(
                            attnT[r0:r0 + 64, hc, s0:s0 + SUB],
                            ps_pv[0:DH, :],
                            recipbc[:, s0:s0 + SUB],
                        )

            prev = None
            for j in range(HEADS // 2):
                st = stage1(j)
                if ci > 0:
                    outproj_sweep(ci - 1, j)
                else:
                    for _ in range(2):
                        if fillers:
                            fillers.pop(0)()
                if prev is not None:
                    stage2(j - 1, prev)
                prev = st
            if ci == 0:
                while fillers:
                    fillers.pop(0)()
            stage2(HEADS // 2 - 1, prev)

        for c in range(KC5):
            outproj_sweep(NCHUNK - 1, c)

    nc.compile()
    return nc


_CACHE = {}


def _get_nc(sig_scale, ag01):
    key = (round(float(sig_scale), 12), round(float(ag01), 12))
    if key not in _CACHE:
        _CACHE[key] = _build(float(sig_scale), float(ag01))
    return _CACHE[key]


def _prior():
    lin = np.linspace(-1.0, 1.0, 64)
    yy, xx = np.meshgrid(lin, lin, indexing="ij")
    g = np.exp(-(xx**2 + yy**2) / (2 * 0.55**2))
    return g.reshape(-1).astype(np.float32)


def kernel(hidden_states, encoder_hidden_states, au_embedding, Wq, Wk, Wv,
           Wau_k, Wau_v, Wout, bout, temperature, au_gate):
    global LAST_EXEC_NS
    bf = ml_dtypes.bfloat16

    hs = np.asarray(hidden_states, dtype=np.float32)
    ehs = np.asarray(encoder_hidden_states, dtype=np.float32)
    au = np.asarray(au_embedding, dtype=np.float32)
    temp = float(np.abs(np.asarray(temperature).reshape(-1)[0])) + 1e-6
    ag = float(np.asarray(au_gate).reshape(-1)[0])

    sig_scale = SCALE / temp
    ag01 = ag * 0.1
    nc = _get_nc(sig_scale, ag01)

    pvec = (ag * 0.9 * _prior()).reshape(1, S).astype(bf)
    shared = {
        "wq": np.asarray(Wq, np.float32).astype(bf),
        "wk": np.asarray(Wk, np.float32).astype(bf),
        "wv": np.asarray(Wv, np.float32).astype(bf),
        "wauk": np.asarray(Wau_k, np.float32).astype(bf),
        "wauv": np.asarray(Wau_v, np.float32).astype(bf),
        "wout": np.asarray(Wout, np.float32).astype(bf),
        "pv": pvec,
        "bvecT": np.asarray(bout, np.float32).reshape(KC5, 128).T.copy(),
    }
    in_maps = []
    for b in range(B):
        m = dict(shared)
        hsT = np.ascontiguousarray(hs[b].T)
        m["hsT"] = hsT.astype(bf)
        m["hsrT"] = hsT
        m["ehsT"] = np.ascontiguousarray(ehs[b].T).astype(bf)
        m["auT"] = np.ascontiguousarray(au[b].T).astype(bf)
        in_maps.append(m)

    trace = bool(os.environ.get("KERNEL_TRACE"))
    if trace:
        try:
            import trace_shim
            trace_shim.install()
        except Exception:
            pass
    res = run_bass_kernel_spmd(nc, in_maps, core_ids=list(range(NCORES)), trace=trace)
    LAST_EXEC_NS = res.exec_time_ns

    out = np.stack([res.results[i]["y"].T for i in range(B)]).astype(np.float32)
    return np.ascontiguousarray(out)
